# revision 5
# baseline (speedup 1.0000x reference)
"""Trainium2 Bass kernel for CrossAttention (B=2, T=S=2048, E=1024, H=16, D=64).

Sharding: 8 cores = 2 (batch) x 4 (head groups of 4 heads).
Each core computes, for its (b, g):
  - Q/K projections in feature-major layout: QT/KT = [256, 2048]
  - V projection in sequence-major layout with an appended ones column per
    head (softmax denominator comes free from the attn@V matmul)
  - causal flash-style attention:
      scores matmuls (full-width) -> additive causal mask folded into PSUM via
      a tiny identity-lhsT matmul on the 128-wide diagonal boundary block ->
      one big grouped exp on ScalarE (2 s-blocks x 2 heads = 2048 cols per
      ACTIVATE to amortize the ~352-cycle instruction overhead) ->
      causally-restricted attn@V accumulation
  - output projection partial: [1024, 2048] fp16, DMA'd out per chunk
Schedule: j-outer software pipeline; Q/K/V projections of chunk j+1 and the
o-projection of chunk j-1 are drained as background PE work between attention
groups so the PE never idles (keeps HAM at K=8/8 / 2.4 GHz).
Host: shards/transposes inputs, gathers partials, sums 4 groups per batch,
adds bo.
"""

import collections

import ml_dtypes
import numpy as np

import concourse.bass as bass
import concourse.bacc as bacc
import concourse.mybir as mybir
import concourse.tile as tile
from concourse.bass_utils import run_bass_kernel_spmd

P = 128
T = 2048          # target length
S = 2048          # source length
E = 1024          # embed dim
D = 64            # head dim
GC = 256          # channels per group (4 heads * 64)
KB = E // P       # 8 k-blocks for the E contraction
TJ = 512          # t-chunk width
NTJ = T // TJ     # 4
NSB = S // P      # 16 s-blocks
VC = 4 * (D + 1)  # 260 = V-projection cols (64 V + 1 ones per head)
SCALE = float(D) ** -0.5  # 0.125
NEG = -60000.0    # additive causal-mask constant (exp(SCALE*NEG) == 0)

F32 = mybir.dt.float32
F16 = mybir.dt.float16


def _build_program(padded: bool):
    nc = bacc.Bacc()

    xq = nc.dram_tensor("xq_t", [E, T], F16, kind="ExternalInput")
    xk = nc.dram_tensor("xk_t", [E, S], F16, kind="ExternalInput")
    xv = nc.dram_tensor("xv_t", [E, S], F16, kind="ExternalInput")
    wq = nc.dram_tensor("wq_t", [E, GC], F16, kind="ExternalInput")
    wk = nc.dram_tensor("wk_t", [E, GC], F16, kind="ExternalInput")
    wv = nc.dram_tensor("wv_t", [E + 1, VC], F16, kind="ExternalInput")
    wo = nc.dram_tensor("wo_t", [GC, E], F16, kind="ExternalInput")
    # itri: [:, :128] identity, [:, 128:] strictly-lower-tri * NEG
    itri = nc.dram_tensor("itri", [P, 2 * P], F16, kind="ExternalInput")
    # padb: cols 0..15 per-s-block exp bias (0 / -1e30), cols 16..19 qk biases
    padb = nc.dram_tensor("padb", [P, NSB + 4], F32, kind="ExternalInput")
    out_t = nc.dram_tensor("out_t", [E, T], F16, kind="ExternalOutput")

    with tile.TileContext(nc) as tc:
        with (
            tc.tile_pool(name="consts", bufs=1) as cpool,
            tc.tile_pool(name="xs", bufs=6) as xpool,
            tc.tile_pool(name="persist", bufs=1) as ppool,
            tc.tile_pool(name="expw", bufs=3) as epool,
            tc.tile_pool(name="norm", bufs=4) as npool,
            tc.tile_pool(name="ft", bufs=4) as fpool,
            tc.tile_pool(name="ps", bufs=1, space="PSUM") as pspool,
        ):
            # ---- constants / weights to SBUF ----
            wq_sb = cpool.tile([P, KB, GC], F16, name="wq_sb")
            wk_sb = cpool.tile([P, KB, GC], F16, name="wk_sb")
            wv_sb = cpool.tile([P, KB + 1, VC], F16, name="wv_sb")
            wo_sb = cpool.tile([P, 2, E], F16, name="wo_sb")
            itri_sb = cpool.tile([P, 2 * P], F16, name="itri_sb")
            padb_sb = cpool.tile([P, NSB + 4], F32, name="padb_sb")
            ones_sb = cpool.tile([1, P], F16, name="ones_sb")

            nc.sync.dma_start(wk_sb[:], wk.rearrange("(kb p) c -> p kb c", p=P))
            nc.sync.dma_start(wq_sb[:], wq.rearrange("(kb p) c -> p kb c", p=P))
            nc.sync.dma_start(
                wv_sb[:, :KB, :],
                wv[: KB * P, :].rearrange("(kb p) c -> p kb c", p=P),
            )
            nc.sync.dma_start(wv_sb[0:1, KB, :], wv[KB * P : KB * P + 1, :])
            nc.sync.dma_start(wo_sb[:], wo.rearrange("(cc p) o -> p cc o", p=P))
            nc.sync.dma_start(itri_sb[:], itri[:])
            nc.sync.dma_start(padb_sb[:], padb[:])
            nc.vector.memset(ones_sb[:], 1.0)

            # ---- persistent activations ----
            qt_sb = ppool.tile([P, 2, T], F16, name="qt_sb")
            kt_sb = ppool.tile([P, 2, S], F16, name="kt_sb")
            v_sb = ppool.tile([P, NSB, VC], F16, name="v_sb")
            aoTn = ppool.tile([P, 2, T], F16, name="aoTn")

            xts = {}

            def load_x(j):
                # one DMA per (tensor, chunk): [128, 8, 512] fp16 (1MB)
                for nm, x_dram in (("k", xk), ("q", xq), ("v", xv)):
                    t_ = xpool.tile([P, KB, TJ], F16, tag="xs", name=f"x{nm}")
                    nc.sync.dma_start(
                        t_[:],
                        x_dram.rearrange("(kb p) t -> p kb t", p=P)[
                            :, :, j * TJ : (j + 1) * TJ
                        ],
                    )
                    xts[(nm, j)] = t_

            # ---------- background task machinery ----------
            bg = collections.deque()

            def drain(n):
                for _ in range(n):
                    if bg:
                        bg.popleft()()

            def qk_proj_tasks(j):
                # channel-major Q/K projections for chunk j
                tasks = []
                for ti, (nm, w_sb, dst, bcol) in enumerate(
                    (("k", wk_sb, kt_sb, NSB), ("q", wq_sb, qt_sb, NSB + 2))
                ):
                    for mc in range(2):
                        st = {}

                        def open_ps(st=st):
                            st["ps"] = pspool.tile(
                                [P, TJ], F32, tag="pr", name="ps_pr", bufs=2
                            )

                        def mm(kb0, st=st, nm=nm, w_sb=w_sb, mc=mc, j=j):
                            xt = xts[(nm, j)]
                            for kb in (kb0, kb0 + 1):
                                nc.tensor.matmul(
                                    st["ps"][:],
                                    lhsT=w_sb[:, kb, mc * P : (mc + 1) * P],
                                    rhs=xt[:, kb, :],
                                    start=(kb == 0),
                                    stop=(kb == KB - 1),
                                )

                        def fin(st=st, dst=dst, mc=mc, j=j, bcol=bcol):
                            nc.vector.tensor_scalar_add(
                                dst[:, mc, j * TJ : (j + 1) * TJ],
                                st["ps"][:],
                                padb_sb[:, bcol + mc : bcol + mc + 1],
                            )

                        tasks.append(open_ps)
                        for kb0 in range(0, KB, 2):
                            tasks.append(lambda kb0=kb0, mm=mm: mm(kb0))
                        tasks.append(fin)
                return tasks

            def v_proj_tasks(j):
                tasks = []
                for ii in range(TJ // P):
                    i = j * (TJ // P) + ii
                    st = {}

                    def open_ps(st=st):
                        st["ps"] = pspool.tile(
                            [P, TJ], F32, tag="pr", name="ps_v", bufs=2
                        )

                    def mm(kb0, st=st, ii=ii, j=j):
                        xt = xts[("v", j)]
                        for kb in (kb0, kb0 + 1):
                            nc.tensor.matmul(
                                st["ps"][:, :VC],
                                lhsT=xt[:, kb, ii * P : (ii + 1) * P],
                                rhs=wv_sb[:, kb, :],
                                start=(kb == 0),
                                stop=False,
                            )

                    def fin(st=st, i=i):
                        nc.tensor.matmul(
                            st["ps"][:, :VC],
                            lhsT=ones_sb[0:1, 0:P],
                            rhs=wv_sb[0:1, KB, :],
                            start=False,
                            stop=True,
                        )
                        nc.vector.tensor_copy(
                            out=v_sb[:, i, :], in_=st["ps"][:, :VC]
                        )

                    tasks.append(open_ps)
                    for kb0 in range(0, KB, 2):
                        tasks.append(lambda kb0=kb0, mm=mm: mm(kb0))
                    tasks.append(fin)
                return tasks

            def o_proj_tasks(j):
                tasks = []
                jsl = slice(j * TJ, (j + 1) * TJ)
                for mc in range(KB):
                    def task(mc=mc, jsl=jsl, j=j):
                        ps = pspool.tile([P, TJ], F32, tag="pr", name="ps_o", bufs=2)
                        for cc in range(2):
                            nc.tensor.matmul(
                                ps[:],
                                lhsT=wo_sb[:, cc, mc * P : (mc + 1) * P],
                                rhs=aoTn[:, cc, jsl],
                                start=(cc == 0),
                                stop=(cc == 1),
                            )
                        oc = fpool.tile([P, TJ], F16, tag="oc", name="oc", bufs=4)
                        nc.vector.tensor_copy(out=oc[:], in_=ps[:])
                        nc.gpsimd.dma_start(
                            out_t[mc * P : (mc + 1) * P, jsl], oc[:]
                        )
                    tasks.append(task)
                return tasks

            # ---------- attention ----------
            # exp grouping: padded=False -> 2 s-blocks x 2 heads per ACTIVATE
            # (2048 cols, constant bias 0); padded=True -> 1 s-block x 2 heads
            # per ACTIVATE with the per-s-block padding bias vector.
            G = 1 if padded else 2
            NSL = 2 * G  # psum slots per scores group

            def attention(hp, j):
                nsb_j = 4 * j + 4
                ngrp = nsb_j // G
                jsl = slice(j * TJ, (j + 1) * TJ)
                av_ps = [
                    pspool.tile([P, TJ], F32, tag=f"av{lh}", name="ps_av", bufs=1)
                    for lh in range(2)
                ]
                ets = {}

                def emit_scores(m):
                    ps = pspool.tile(
                        [P, NSL, TJ], F32, tag="sc", name="ps_sc",
                        bufs=(2 if padded else 1),
                    )
                    for u in range(G):
                        i = G * m + u
                        r = i - 4 * j
                        for lh in range(2):
                            base = D * lh
                            nc.tensor.matmul(
                                ps[:, 2 * u + lh, :],
                                lhsT=kt_sb[base : base + D, hp, i * P : (i + 1) * P],
                                rhs=qt_sb[base : base + D, hp, jsl],
                                start=True,
                                stop=(r < 0),
                                skip_group_check=True,
                            )
                            if r >= 0:
                                # fold causal mask additively into PSUM:
                                # ps[:, u, ci:ci+128] += I.T @ (NEG * lowtri)
                                ci = r * P
                                nc.tensor.matmul(
                                    ps[:, 2 * u + lh, ci : ci + P],
                                    lhsT=itri_sb[:, 0:P],
                                    rhs=itri_sb[:, P : 2 * P],
                                    start=False,
                                    stop=True,
                                    skip_group_check=True,
                                )
                    return ps

                def emit_exp(m, ps):
                    et = epool.tile([P, NSL, TJ], F16, tag="exp", name="et")
                    if padded:
                        i = m  # G == 1
                        nc.scalar.activation(
                            et[:],
                            ps[:],
                            mybir.ActivationFunctionType.Exp,
                            scale=SCALE,
                            bias=padb_sb[:, i : i + 1],
                        )
                    else:
                        nc.scalar.activation(
                            et[:],
                            ps[:],
                            mybir.ActivationFunctionType.Exp,
                            scale=SCALE,
                        )
                    ets[m] = et

                def emit_av(m):
                    et = ets.pop(m)
                    for u in range(G):
                        i = G * m + u
                        ci = max(0, i - 4 * j) * P
                        for lh in range(2):
                            h65 = (hp * 2 + lh) * (D + 1)
                            nc.tensor.matmul(
                                av_ps[lh][: D + 1, ci:TJ],
                                lhsT=v_sb[:, i, h65 : h65 + D + 1],
                                rhs=et[:, 2 * u + lh, ci:TJ],
                                start=(i == 0),
                                stop=(i == nsb_j - 1),
                                skip_group_check=True,
                            )

                # software pipeline: sc(m) -> exp(m) -> [sc(m+1)] -> av(m)
                ps_prev = emit_scores(0)
                for m in range(ngrp):
                    emit_exp(m, ps_prev)
                    drain(2)
                    if m + 1 < ngrp:
                        ps_prev = emit_scores(m + 1)
                    emit_av(m)
                    drain(2)

                # normalize: row D of av psum is the softmax denominator
                for lh in range(2):
                    aoF = npool.tile([D + 1, TJ], F32, tag="aoF", name="aoF", bufs=2)
                    nc.vector.tensor_copy(out=aoF[:], in_=av_ps[lh][: D + 1, :])
                    rrow = npool.tile([1, TJ], F32, tag="rrow", name="rrow", bufs=3)
                    # partition shift 64 -> 0 via SBUF DMA (PSUM is not DMA-able)
                    nc.gpsimd.dma_start(rrow[:], aoF[D : D + 1, :])
                    rcp = npool.tile([1, TJ], F32, tag="rcp", name="rcp", bufs=3)
                    nc.vector.reciprocal_approx_fast(rcp[:], rrow[:])
                    rb64 = npool.tile([D, TJ], F32, tag="rb64", name="rb64", bufs=3)
                    nc.gpsimd.partition_broadcast(rb64[:], rcp[0:1, :])
                    if lh == 0:
                        nc.vector.tensor_mul(
                            out=aoTn[0:D, hp, jsl],
                            in0=aoF[0:D, :],
                            in1=rb64[:],
                        )
                    else:
                        tmp = npool.tile([D, TJ], F16, tag="aon", name="aon", bufs=3)
                        nc.vector.tensor_mul(
                            out=tmp[:], in0=aoF[0:D, :], in1=rb64[:]
                        )
                        # partition shift 0-63 -> 64-127 via SBUF DMA
                        nc.gpsimd.dma_start(aoTn[D : 2 * D, hp, jsl], tmp[:])

            # ---------- main schedule ----------
            load_x(0)
            for t in qk_proj_tasks(0):
                t()
            load_x(1)
            for t in v_proj_tasks(0):
                t()
            for j in range(NTJ):
                # all proj(j) tasks must be emitted before attention(j) reads
                # qt/kt/v (the PE executes its queue in order)
                drain(len(bg))
                if j + 1 < NTJ:
                    if j + 2 < NTJ:
                        bg.append(lambda jn=j + 2: load_x(jn))
                    for t in qk_proj_tasks(j + 1):
                        bg.append(t)
                    for t in v_proj_tasks(j + 1):
                        bg.append(t)
                attention(0, j)
                attention(1, j)
                for t in o_proj_tasks(j):
                    bg.append(t)
            drain(len(bg))

    nc.compile()
    return nc


_NC_CACHE = {}


def _get_nc(padded: bool = False):
    if padded not in _NC_CACHE:
        _NC_CACHE[padded] = _build_program(padded)
    return _NC_CACHE[padded]


def _make_in_maps(query, key, value, key_padding_mask, Wq, bq, Wk, bk, Wv, bv, Wo, bo):
    f32 = np.float32
    f16 = np.float16
    query = np.asarray(query, f32)
    key = np.asarray(key, f32)
    value = np.asarray(value, f32)
    kpm = np.asarray(key_padding_mask, bool)
    Wq, bq = np.asarray(Wq, f32), np.asarray(bq, f32)
    Wk, bk = np.asarray(Wk, f32), np.asarray(bk, f32)
    Wv, bv = np.asarray(Wv, f32), np.asarray(bv, f32)
    Wo = np.asarray(Wo, f32)

    # constants shared by all cores: identity | strictly-lower-tri * NEG
    ident = np.eye(P, dtype=f16)
    lowtri = (np.arange(P)[None, :] < np.arange(P)[:, None]).astype(f16) * f16(NEG)
    itri_np = np.concatenate([ident, lowtri], axis=1)

    in_maps = []
    for c in range(8):
        b, g = divmod(c, 4)
        cols = slice(g * GC, (g + 1) * GC)

        wq_t = Wq[cols, :].T.astype(f16)
        wk_t = Wk[cols, :].T.astype(f16)

        wv_t = np.zeros((E + 1, VC), f16)
        for h in range(4):
            ch = slice(g * GC + h * D, g * GC + (h + 1) * D)
            wv_t[:E, h * (D + 1) : h * (D + 1) + D] = Wv[ch, :].T
            wv_t[E, h * (D + 1) : h * (D + 1) + D] = bv[ch]
            wv_t[E, h * (D + 1) + D] = 1.0  # ones column -> softmax denominator

        wo_t = np.ascontiguousarray(Wo[:, cols].T.astype(f16))

        padb_np = np.where(kpm[b], -1.0e30, 0.0).astype(f32).reshape(NSB, P).T
        biases = np.stack(
            [bk[cols][:P], bk[cols][P:], bq[cols][:P], bq[cols][P:]], axis=1
        ).astype(f32)
        padb_np = np.ascontiguousarray(np.concatenate([padb_np, biases], axis=1))

        in_maps.append(
            {
                "xq_t": np.ascontiguousarray(query[b].T.astype(f16)),
                "xk_t": np.ascontiguousarray(key[b].T.astype(f16)),
                "xv_t": np.ascontiguousarray(value[b].T.astype(f16)),
                "wq_t": np.ascontiguousarray(wq_t),
                "wk_t": np.ascontiguousarray(wk_t),
                "wv_t": wv_t,
                "wo_t": wo_t,
                "itri": itri_np,
                "padb": padb_np,
            }
        )
    return in_maps


def kernel(**inputs) -> np.ndarray:
    padded = bool(np.asarray(inputs["key_padding_mask"]).any())
    nc = _get_nc(padded)
    in_maps = _make_in_maps(**inputs)
    res = run_bass_kernel_spmd(nc, in_maps, core_ids=list(range(8)))
    bo = np.asarray(inputs["bo"], np.float32)
    B = inputs["query"].shape[0]
    out = np.zeros((B, T, E), np.float32)
    for c in range(8):
        b = c // 4
        out[b] += res.results[c]["out_t"].T.astype(np.float32)
    out += bo[None, None, :]
    return out


# revision 6
# speedup vs baseline: 1.0743x; 1.0743x over previous
"""Trainium2 Bass kernel for CrossAttention (B=2, T=S=2048, E=1024, H=16, D=64).

Sharding: 8 cores = 2 (batch) x 4 (head groups of 4 heads).
Each core computes, for its (b, g):
  - Q/K projections in feature-major layout: QT/KT = [256, 2048]
  - V projection in sequence-major layout with an appended ones column per
    head (softmax denominator comes free from the attn@V matmul)
  - causal flash-style attention:
      causally-restricted scores matmuls -> additive causal mask folded into
      PSUM via a tiny identity-lhsT matmul on the 128-wide diagonal boundary
      block -> one exp per s-block covering both heads (1024 cols per
      ACTIVATE; key-padding handled by the per-partition exp bias) ->
      causally-restricted attn@V accumulation
  - output projection partial: [1024, 2048] fp16, DMA'd out per chunk
Schedule: j-outer software pipeline; Q/K/V projections of chunk j+1 and the
o-projection of chunk j-1 are drained as background PE work between attention
steps so the PE never idles (keeps HAM at K=8/8 / 2.4 GHz).
Host: shards/transposes inputs, gathers partials, sums 4 groups per batch,
adds bo.
"""

import collections

import ml_dtypes
import numpy as np

import concourse.bass as bass
import concourse.bacc as bacc
import concourse.mybir as mybir
import concourse.tile as tile
from concourse.bass_utils import run_bass_kernel_spmd

P = 128
T = 2048          # target length
S = 2048          # source length
E = 1024          # embed dim
D = 64            # head dim
GC = 256          # channels per group (4 heads * 64)
KB = E // P       # 8 k-blocks for the E contraction
TJ = 512          # t-chunk width
NTJ = T // TJ     # 4
NSB = S // P      # 16 s-blocks
VC = 4 * (D + 1)  # 260 = V-projection cols (64 V + 1 ones per head)
SCALE = float(D) ** -0.5  # 0.125
NEG = -60000.0    # additive causal-mask constant (exp(SCALE*NEG) == 0)

F32 = mybir.dt.float32
F16 = mybir.dt.float16


def _build_program():
    nc = bacc.Bacc()

    xq = nc.dram_tensor("xq_t", [E, T], F16, kind="ExternalInput")
    xk = nc.dram_tensor("xk_t", [E, S], F16, kind="ExternalInput")
    xv = nc.dram_tensor("xv_t", [E, S], F16, kind="ExternalInput")
    wq = nc.dram_tensor("wq_t", [E, GC], F16, kind="ExternalInput")
    wk = nc.dram_tensor("wk_t", [E, GC], F16, kind="ExternalInput")
    wv = nc.dram_tensor("wv_t", [E + 1, VC], F16, kind="ExternalInput")
    wo = nc.dram_tensor("wo_t", [GC, E], F16, kind="ExternalInput")
    # itri: [:, :128] identity, [:, 128:] strictly-lower-tri * NEG
    itri = nc.dram_tensor("itri", [P, 2 * P], F16, kind="ExternalInput")
    # padb: cols 0..15 per-s-block exp bias (0 / -1e30), cols 16..19 qk biases
    padb = nc.dram_tensor("padb", [P, NSB + 4], F32, kind="ExternalInput")
    out_t = nc.dram_tensor("out_t", [E, T], F16, kind="ExternalOutput")

    with tile.TileContext(nc) as tc:
        with (
            tc.tile_pool(name="consts", bufs=1) as cpool,
            tc.tile_pool(name="x0", bufs=1) as x0pool,
            tc.tile_pool(name="xs", bufs=6) as xpool,
            tc.tile_pool(name="persist", bufs=1) as ppool,
            tc.tile_pool(name="expw", bufs=4) as epool,
            tc.tile_pool(name="norm", bufs=4) as npool,
            tc.tile_pool(name="ft", bufs=4) as fpool,
            tc.tile_pool(name="ps", bufs=1, space="PSUM") as pspool,
        ):
            # ---- constants / weights to SBUF ----
            wq_sb = cpool.tile([P, KB, GC], F16, name="wq_sb")
            wk_sb = cpool.tile([P, KB, GC], F16, name="wk_sb")
            wv_sb = cpool.tile([P, KB + 1, VC], F16, name="wv_sb")
            wo_sb = cpool.tile([P, 2, E], F16, name="wo_sb")
            itri_sb = cpool.tile([P, 2 * P], F16, name="itri_sb")
            padb_sb = cpool.tile([P, NSB + 4], F32, name="padb_sb")
            ones_sb = cpool.tile([1, P], F16, name="ones_sb")

            # first-chunk x tiles are loaded per-kb so the first projection
            # matmuls start ~2us in instead of waiting for 1MB transfers
            x0 = {}
            nc.sync.dma_start(wk_sb[:], wk.rearrange("(kb p) c -> p kb c", p=P))
            for nm, x_dram in (("k", xk), ("q", xq), ("v", xv)):
                tiles = []
                for kb in range(KB):
                    t_ = x0pool.tile([P, TJ], F16, name=f"x0{nm}{kb}")
                    nc.sync.dma_start(
                        t_[:], x_dram[kb * P : (kb + 1) * P, 0:TJ]
                    )
                    tiles.append(t_)
                x0[nm] = tiles
                if nm == "k":
                    nc.sync.dma_start(
                        wq_sb[:], wq.rearrange("(kb p) c -> p kb c", p=P)
                    )
                elif nm == "q":
                    nc.sync.dma_start(
                        wv_sb[:, :KB, :],
                        wv[: KB * P, :].rearrange("(kb p) c -> p kb c", p=P),
                    )
                    nc.sync.dma_start(
                        wv_sb[0:1, KB, :], wv[KB * P : KB * P + 1, :]
                    )
            nc.sync.dma_start(wo_sb[:], wo.rearrange("(cc p) o -> p cc o", p=P))
            nc.sync.dma_start(itri_sb[:], itri[:])
            nc.sync.dma_start(padb_sb[:], padb[:])
            nc.vector.memset(ones_sb[:], 1.0)

            # ---- persistent activations ----
            qt_sb = ppool.tile([P, 2, T], F16, name="qt_sb")
            kt_sb = ppool.tile([P, 2, S], F16, name="kt_sb")
            v_sb = ppool.tile([P, NSB, VC], F16, name="v_sb")
            aoTn = ppool.tile([P, 2, T], F16, name="aoTn")

            xts = {}

            def load_x(j):
                # one DMA per (tensor, chunk): [128, 8, 512] fp16 (1MB)
                for nm, x_dram in (("k", xk), ("q", xq), ("v", xv)):
                    t_ = xpool.tile([P, KB, TJ], F16, tag="xs", name=f"x{nm}")
                    nc.sync.dma_start(
                        t_[:],
                        x_dram.rearrange("(kb p) t -> p kb t", p=P)[
                            :, :, j * TJ : (j + 1) * TJ
                        ],
                    )
                    xts[(nm, j)] = t_

            def xslice(nm, j, kb, c0, c1):
                if j == 0:
                    return x0[nm][kb][:, c0:c1]
                return xts[(nm, j)][:, kb, c0:c1]

            # ---------- background task machinery ----------
            bg = collections.deque()

            def drain(n):
                for _ in range(n):
                    if bg:
                        bg.popleft()()

            def qk_proj_tasks(j):
                # channel-major Q/K projections for chunk j
                tasks = []
                for nm, w_sb, dst, bcol in (
                    ("k", wk_sb, kt_sb, NSB),
                    ("q", wq_sb, qt_sb, NSB + 2),
                ):
                    for mc in range(2):
                        st = {}

                        def open_ps(st=st):
                            st["ps"] = pspool.tile(
                                [P, TJ], F32, tag="pr", name="ps_pr", bufs=2
                            )

                        def mm(kb0, st=st, nm=nm, w_sb=w_sb, mc=mc, j=j):
                            for kb in (kb0, kb0 + 1):
                                nc.tensor.matmul(
                                    st["ps"][:],
                                    lhsT=w_sb[:, kb, mc * P : (mc + 1) * P],
                                    rhs=xslice(nm, j, kb, 0, TJ),
                                    start=(kb == 0),
                                    stop=(kb == KB - 1),
                                )

                        def fin(st=st, dst=dst, mc=mc, j=j, bcol=bcol):
                            nc.vector.tensor_scalar_add(
                                dst[:, mc, j * TJ : (j + 1) * TJ],
                                st["ps"][:],
                                padb_sb[:, bcol + mc : bcol + mc + 1],
                            )

                        tasks.append(open_ps)
                        for kb0 in range(0, KB, 2):
                            tasks.append(lambda kb0=kb0, mm=mm: mm(kb0))
                        tasks.append(fin)
                return tasks

            def v_proj_tasks(j):
                tasks = []
                for ii in range(TJ // P):
                    i = j * (TJ // P) + ii
                    st = {}

                    def open_ps(st=st):
                        st["ps"] = pspool.tile(
                            [P, TJ], F32, tag="pr", name="ps_v", bufs=2
                        )

                    def mm(kb0, st=st, ii=ii, j=j):
                        for kb in (kb0, kb0 + 1):
                            nc.tensor.matmul(
                                st["ps"][:, :VC],
                                lhsT=xslice(nm_v, j, kb, ii * P, (ii + 1) * P),
                                rhs=wv_sb[:, kb, :],
                                start=(kb == 0),
                                stop=False,
                            )

                    def fin(st=st, i=i):
                        nc.tensor.matmul(
                            st["ps"][:, :VC],
                            lhsT=ones_sb[0:1, 0:P],
                            rhs=wv_sb[0:1, KB, :],
                            start=False,
                            stop=True,
                        )
                        nc.vector.tensor_copy(
                            out=v_sb[:, i, :], in_=st["ps"][:, :VC]
                        )

                    nm_v = "v"
                    tasks.append(open_ps)
                    for kb0 in range(0, KB, 2):
                        tasks.append(lambda kb0=kb0, mm=mm: mm(kb0))
                    tasks.append(fin)
                return tasks

            def o_proj_tasks(j):
                tasks = []
                jsl = slice(j * TJ, (j + 1) * TJ)
                for mc in range(KB):
                    def task(mc=mc, jsl=jsl, j=j):
                        ps = pspool.tile([P, TJ], F32, tag="pr", name="ps_o", bufs=2)
                        for cc in range(2):
                            nc.tensor.matmul(
                                ps[:],
                                lhsT=wo_sb[:, cc, mc * P : (mc + 1) * P],
                                rhs=aoTn[:, cc, jsl],
                                start=(cc == 0),
                                stop=(cc == 1),
                            )
                        oc = fpool.tile([P, TJ], F16, tag="oc", name="oc", bufs=4)
                        nc.vector.tensor_copy(out=oc[:], in_=ps[:])
                        eng = nc.sync if mc % 2 == 0 else nc.gpsimd
                        eng.dma_start(out_t[mc * P : (mc + 1) * P, jsl], oc[:])
                    tasks.append(task)
                return tasks

            # ---------- attention ----------
            def attention(hp, j):
                nsb_j = 4 * j + 4
                jsl = slice(j * TJ, (j + 1) * TJ)
                av_ps = [
                    pspool.tile([P, TJ], F32, tag=f"av{lh}", name="ps_av", bufs=1)
                    for lh in range(2)
                ]
                ets = {}

                def ci_of(i):
                    return max(0, i - 4 * j) * P

                def emit_scores(i):
                    ci = ci_of(i)
                    ps = pspool.tile([P, 2, TJ], F32, tag="sc", name="ps_sc", bufs=2)
                    for lh in range(2):
                        base = D * lh
                        nc.tensor.matmul(
                            ps[:, lh, ci:TJ],
                            lhsT=kt_sb[base : base + D, hp, i * P : (i + 1) * P],
                            rhs=qt_sb[base : base + D, hp, j * TJ + ci : (j + 1) * TJ],
                            start=True,
                            stop=(i < 4 * j),
                            skip_group_check=True,
                        )
                        if i >= 4 * j:
                            # fold causal mask additively into PSUM:
                            # ps[:, lh, ci:ci+128] += I.T @ (NEG * lowtri)
                            nc.tensor.matmul(
                                ps[:, lh, ci : ci + P],
                                lhsT=itri_sb[:, 0:P],
                                rhs=itri_sb[:, P : 2 * P],
                                start=False,
                                stop=True,
                                skip_group_check=True,
                            )
                    return ps

                def emit_exp(i, ps):
                    ci = ci_of(i)
                    et = epool.tile([P, 2, TJ], F16, tag="exp", name="et")
                    nc.scalar.activation(
                        et[:, :, ci:TJ],
                        ps[:, :, ci:TJ],
                        mybir.ActivationFunctionType.Exp,
                        scale=SCALE,
                        bias=padb_sb[:, i : i + 1],
                    )
                    ets[i] = et

                def emit_av(i):
                    ci = ci_of(i)
                    et = ets.pop(i)
                    for lh in range(2):
                        h65 = (hp * 2 + lh) * (D + 1)
                        nc.tensor.matmul(
                            av_ps[lh][: D + 1, ci:TJ],
                            lhsT=v_sb[:, i, h65 : h65 + D + 1],
                            rhs=et[:, lh, ci:TJ],
                            start=(i == 0),
                            stop=(i == nsb_j - 1),
                            skip_group_check=True,
                        )

                # software pipeline: sc(i) -> exp(i) -> [sc(i+1)] -> av(i)
                ps_prev = emit_scores(0)
                for i in range(nsb_j):
                    emit_exp(i, ps_prev)
                    drain(1)
                    if i + 1 < nsb_j:
                        ps_prev = emit_scores(i + 1)
                    emit_av(i)
                    drain(1)

                # normalize: row D of av psum is the softmax denominator.
                # DVE ops are batched pairwise so FIFO heads never wait DMA.
                aoF = [
                    npool.tile([D + 1, TJ], F32, tag=f"aoF{lh}", name="aoF", bufs=2)
                    for lh in range(2)
                ]
                rrow = [
                    npool.tile([1, TJ], F32, tag=f"rrow{lh}", name="rrow", bufs=2)
                    for lh in range(2)
                ]
                rcp = [
                    npool.tile([1, TJ], F32, tag=f"rcp{lh}", name="rcp", bufs=2)
                    for lh in range(2)
                ]
                rb64 = [
                    npool.tile([D, TJ], F32, tag=f"rb{lh}", name="rb64", bufs=2)
                    for lh in range(2)
                ]
                for lh in range(2):
                    nc.vector.tensor_copy(
                        out=aoF[lh][:], in_=av_ps[lh][: D + 1, :]
                    )
                for lh in range(2):
                    # partition shift 64 -> 0 via SBUF DMA (PSUM not DMA-able)
                    nc.gpsimd.dma_start(rrow[lh][:], aoF[lh][D : D + 1, :])
                for lh in range(2):
                    nc.vector.reciprocal_approx_fast(rcp[lh][:], rrow[lh][:])
                for lh in range(2):
                    nc.gpsimd.partition_broadcast(rb64[lh][:], rcp[lh][0:1, :])
                nc.vector.tensor_mul(
                    out=aoTn[0:D, hp, jsl], in0=aoF[0][0:D, :], in1=rb64[0][:]
                )
                tmp = npool.tile([D, TJ], F16, tag="aon", name="aon", bufs=2)
                nc.vector.tensor_mul(out=tmp[:], in0=aoF[1][0:D, :], in1=rb64[1][:])
                # partition shift 0-63 -> 64-127 via SBUF DMA
                nc.gpsimd.dma_start(aoTn[D : 2 * D, hp, jsl], tmp[:])

            # ---------- main schedule ----------
            for t in qk_proj_tasks(0):
                t()
            load_x(1)
            for t in v_proj_tasks(0):
                t()
            for j in range(NTJ):
                # all proj(j) tasks must be emitted before attention(j) reads
                # qt/kt/v (the PE executes its queue in order)
                drain(len(bg))
                if j + 2 < NTJ:
                    load_x(j + 2)
                if j + 1 < NTJ:
                    for t in qk_proj_tasks(j + 1):
                        bg.append(t)
                    for t in v_proj_tasks(j + 1):
                        bg.append(t)
                attention(0, j)
                attention(1, j)
                for t in o_proj_tasks(j):
                    bg.append(t)
            drain(len(bg))

    nc.compile()
    return nc


_NC_CACHE = None


def _get_nc():
    global _NC_CACHE
    if _NC_CACHE is None:
        _NC_CACHE = _build_program()
    return _NC_CACHE


def _make_in_maps(query, key, value, key_padding_mask, Wq, bq, Wk, bk, Wv, bv, Wo, bo):
    f32 = np.float32
    f16 = np.float16
    query = np.asarray(query, f32)
    key = np.asarray(key, f32)
    value = np.asarray(value, f32)
    kpm = np.asarray(key_padding_mask, bool)
    Wq, bq = np.asarray(Wq, f32), np.asarray(bq, f32)
    Wk, bk = np.asarray(Wk, f32), np.asarray(bk, f32)
    Wv, bv = np.asarray(Wv, f32), np.asarray(bv, f32)
    Wo = np.asarray(Wo, f32)

    # constants shared by all cores: identity | strictly-lower-tri * NEG
    ident = np.eye(P, dtype=f16)
    lowtri = (np.arange(P)[None, :] < np.arange(P)[:, None]).astype(f16) * f16(NEG)
    itri_np = np.concatenate([ident, lowtri], axis=1)

    in_maps = []
    for c in range(8):
        b, g = divmod(c, 4)
        cols = slice(g * GC, (g + 1) * GC)

        wq_t = Wq[cols, :].T.astype(f16)
        wk_t = Wk[cols, :].T.astype(f16)

        wv_t = np.zeros((E + 1, VC), f16)
        for h in range(4):
            ch = slice(g * GC + h * D, g * GC + (h + 1) * D)
            wv_t[:E, h * (D + 1) : h * (D + 1) + D] = Wv[ch, :].T
            wv_t[E, h * (D + 1) : h * (D + 1) + D] = bv[ch]
            wv_t[E, h * (D + 1) + D] = 1.0  # ones column -> softmax denominator

        wo_t = np.ascontiguousarray(Wo[:, cols].T.astype(f16))

        padb_np = np.where(kpm[b], -1.0e30, 0.0).astype(f32).reshape(NSB, P).T
        biases = np.stack(
            [bk[cols][:P], bk[cols][P:], bq[cols][:P], bq[cols][P:]], axis=1
        ).astype(f32)
        padb_np = np.ascontiguousarray(np.concatenate([padb_np, biases], axis=1))

        in_maps.append(
            {
                "xq_t": np.ascontiguousarray(query[b].T.astype(f16)),
                "xk_t": np.ascontiguousarray(key[b].T.astype(f16)),
                "xv_t": np.ascontiguousarray(value[b].T.astype(f16)),
                "wq_t": np.ascontiguousarray(wq_t),
                "wk_t": np.ascontiguousarray(wk_t),
                "wv_t": wv_t,
                "wo_t": wo_t,
                "itri": itri_np,
                "padb": padb_np,
            }
        )
    return in_maps


def kernel(**inputs) -> np.ndarray:
    nc = _get_nc()
    in_maps = _make_in_maps(**inputs)
    res = run_bass_kernel_spmd(nc, in_maps, core_ids=list(range(8)))
    bo = np.asarray(inputs["bo"], np.float32)
    B = inputs["query"].shape[0]
    out = np.zeros((B, T, E), np.float32)
    for c in range(8):
        b = c // 4
        out[b] += res.results[c]["out_t"].T.astype(np.float32)
    out += bo[None, None, :]
    return out
